# revision 7
# baseline (speedup 1.0000x reference)
"""KANLinear TRN2 Bass kernel (8-core SPMD, token-data-parallel).

Math (matches the jax reference, up to fp rounding):
  y[b,o] = silu(x)[b,:] @ scale_base.T  +  sum_{i,g} B_g(x[b,i]) * w[o,i,g]
with cubic B-spline bases on the uniform grid t_j = -1.75 + 0.25*j.

Basis evaluation uses the bounded symmetric form (exact identity):
  a   = |4x + 5 - g|
  m   = relu(2 - a), n = relu(1 - a)
  6*B_g(x) = m^3 - 4*n^3
computed in doubled variables on device:
  a2  = |8x + 2(5-g)|            (ACT Abs, per-channel bias tile)
  u2  = min(a2, 4) - 4  = -2m    (DVE tensor_scalar, 4x mode)
  b   = KAN_BSPLINE(u2) = min(u2+2, 0)^3 - u2^3/4 = 2m^3 - 8n^3 = 12*B
The last step is a runtime-registered custom DVE micro-op (8 ALU
stages, 1 elem/cycle) writing fp8e4 directly, so the whole basis
pipeline is one ACT op per channel + two DVE ops per channel block.

Both matmul families run as fp8 DoubleRow (K=256/instr, 2x PE rate):
the spline einsum (bases x host-scaled coeff) and the silu base
matmul (fp8 silu x fp8 sbt, k-paired over the unit's two in-tiles),
accumulating into shared PSUM banks host-scaled by WSCALE=256 (kept
under fp8e4's 240 max) and drained with 1/WSCALE.

Basis work is chunked by channel pairs so the PE can start each
unit's DoubleRow matmuls as soon as the first two channels convert.
A dozen zero-contribution warmup matmuls ramp the PE p-state during
the DMA lead-in (they also carry the half-0 PSUM start flags), and a
dummy activation pulls ACT_TABLE_LOAD off the silu critical path.
The last unit of each half runs m-major with its (ACT|DVE)-split
drains interleaved per token tile, so drains overlap the remaining
matmuls and the half boundary / kernel tail shrink to one drain.
"""

import numpy as np
import ml_dtypes

import concourse.bass as bass
import concourse.mybir as mybir
import concourse.tile as tile
from concourse import bacc
from concourse.alu_op_type import AluOpType as A
from concourse.bass_utils import run_bass_kernel_spmd

AF = mybir.ActivationFunctionType
F32 = mybir.dt.float32
F16 = mybir.dt.float16
BF16 = mybir.dt.bfloat16
F8 = mybir.dt.float8e4
DR = mybir.MatmulPerfMode.DoubleRow

# problem constants (hardcoded per the task contract)
TOKENS, IN_DIM, OUT_DIM = 8192, 1024, 1024
NB = 11  # cubic B-spline bases per input dim (grid_size + k)
N_CORES = 8
TPC = TOKENS // N_CORES  # tokens per core (1024)
HALF = 512  # tokens per processing chunk (PSUM-bank limited)
M_TILES = HALF // 128  # token tiles per half (4)
N_OC = OUT_DIM // 512  # out-dim chunks (2)
WOC = NB * 512  # weight free size per (it, oc) chunk (5632)
WSCALE = 256.0  # accumulation scale; keeps fp8 sbt under e4m3's 240 max
BSCALE = 12.0  # device basis = 12*B
N_WARM = 15  # PE p-state warmup matmuls

UNITS = [(0, 1), (2, 3), (4, 5), (6, 7)]
J = 2  # in-dim tiles per unit
# channel chunks: pairs for DoubleRow; last chunk carries (8,9) + g10
CH_CHUNKS = [(0, 2), (2, 4), (4, 6), (6, 8), (8, 11)]

_CACHED = None


def _register_kan_op():
    """Register the fused B-spline custom DVE op (idempotent)."""
    import concourse.dve_ops as dom
    from concourse.dve_ops import DveOp
    from concourse.dve_spec import Spec, Src0, Zero, C0, C1, minn, sq, lower
    from concourse.dve_uop import DveOpSpec
    if "KAN_BSPLINE" in dom._SUB_OPCODE_FOR_NAME:
        return next(op for op in dom.OPS if op.name == "KAN_BSPLINE")

    w = minn(Src0 + C0, Zero)
    body = (sq(w) * w) - (sq(Src0) * Src0) * C1

    def ref(in0, in1, s0, s1, imm2):
        x = np.asarray(in0, np.float32)
        wv = np.minimum(x + s0, 0.0)
        return wv ** 3 - (x ** 3) * s1

    spec = Spec(body=body, reference=ref)
    row = max(dom._SUB_OPCODE_FOR_NAME.values()) + 1
    dom._SUB_OPCODE_FOR_NAME["KAN_BSPLINE"] = row
    shas = {}
    for ver in ("v3", "v4"):
        s = DveOpSpec(name="KAN_BSPLINE", opcode=row,
                      uops=lower(spec, ver=ver), rd1_en=False)
        shas[ver] = s.sha(ver)
    op = DveOp("KAN_BSPLINE", spec, subdim=False, uops_sha=shas)
    dom.OPS.append(op)
    dom.CUSTOM_DVE_SPECS["KAN_BSPLINE"] = spec
    return op


def _build_bass():
    kan_op = _register_kan_op()
    nc = bacc.Bacc("TRN2", target_bir_lowering=False, debug=False,
                   num_devices=N_CORES)
    xt = nc.declare_dram_parameter("xt", [IN_DIM, TPC], F16, isOutput=False)
    w8 = nc.declare_dram_parameter("w8", [IN_DIM, NB * OUT_DIM], F8,
                                   isOutput=False)
    sbt = nc.declare_dram_parameter("sbt", [IN_DIM, OUT_DIM], F8,
                                    isOutput=False)
    y = nc.declare_dram_parameter("y", [TPC, OUT_DIM], F32, isOutput=True)

    with tile.TileContext(nc) as tc:
        with (
            tc.tile_pool(name="xts", bufs=4) as xpool,
            tc.tile_pool(name="silu", bufs=2) as spool,
            tc.tile_pool(name="sbts", bufs=8) as sbpool,
            tc.tile_pool(name="a2", bufs=1) as a2pool,
            tc.tile_pool(name="u2", bufs=1) as u2pool,
            tc.tile_pool(name="b8b", bufs=2) as b8pool,
            tc.tile_pool(name="w8t", bufs=4) as w8pool,
            tc.tile_pool(name="outs", bufs=8) as opool,
            tc.tile_pool(name="consts", bufs=1) as kpool,
            tc.tile_pool(name="psum", bufs=8, space="PSUM") as ppool,
        ):
            # per-channel bias constants for a2_g = |8x + 2(5-g)|
            bias_tile = kpool.tile([128, NB], F32, tag="bias")
            for g in range(NB):
                nc.vector.memset(bias_tile[:, g:g + 1], float(2 * (5 - g)))
            # PE warmup operands (zero contribution) + dummy activation
            # to hoist ACT_TABLE_LOAD off the first silu's critical path
            zl = kpool.tile([128, 128], BF16, tag="wzl")
            zr = kpool.tile([128, 512], BF16, tag="wzr")
            nc.vector.memset(zl, 0.0)
            nc.vector.memset(zr, 0.0)
            dumb = kpool.tile([128, 1], F16, tag="dumb")
            nc.scalar.activation(dumb, bias_tile[:, 0:1], AF.Silu)

            xps = {}
            sbtts_all = {}

            def emit_unit_dmas(ui, unit):
                # x and sbt are shared by both halves: load once, with
                # full-row (2KB-burst) descriptors
                xp = xpool.tile([128, J, TPC], F16, tag="xt")
                for j in range(J):
                    it = unit[j]
                    nc.sync.dma_start(
                        out=xp[:, j, :],
                        in_=xt[it * 128:(it + 1) * 128, :])
                xps[ui] = xp
                row = []
                for oc in range(N_OC):
                    sbtt = sbpool.tile([128, J, 512], F8, tag="sbt")
                    for j in range(J):
                        it = unit[j]
                        nc.sync.dma_start(
                            out=sbtt[:, j, :],
                            in_=sbt[it * 128:(it + 1) * 128,
                                    oc * 512:(oc + 1) * 512])
                    row.append(sbtt)
                sbtts_all[ui] = row

            def emit_elementwise(half, ui):
                """abs/silu acts, u2/kan chunks. Returns tiles."""
                t0 = half * HALF
                xph = xps[ui][:, :, t0:t0 + HALF]
                sbtts = sbtts_all[ui]
                # basis chain, chunked by channel group; first two abs
                # channels precede the silu so DVE can start early
                a2 = a2pool.tile([128, NB, J, HALF], F16, tag="a2")
                for g in range(2):
                    nc.scalar.activation(a2[:, g, :, :], xph, AF.Abs,
                                         bias=bias_tile[:, g:g + 1],
                                         scale=8.0)
                sp = spool.tile([128, J, HALF], F8, tag="silu")
                nc.scalar.activation(sp, xph, AF.Silu)
                for g in range(2, NB):
                    nc.scalar.activation(a2[:, g, :, :], xph, AF.Abs,
                                         bias=bias_tile[:, g:g + 1],
                                         scale=8.0)
                u2 = u2pool.tile([128, NB, J, HALF], F16, tag="u2")
                b8 = b8pool.tile([128, NB, J, HALF], F8, tag="b8")
                for (c0, c1) in CH_CHUNKS:
                    nc.vector.tensor_scalar(u2[:, c0:c1, :, :],
                                            a2[:, c0:c1, :, :],
                                            4.0, 4.0, A.min, A.subtract)
                    for j in range(J):
                        nc.vector._custom_dve(
                            kan_op, out=b8[:, c0:c1, j, :],
                            in0=u2[:, c0:c1, j, :], s0=2.0, s1=0.25)
                return sp, sbtts, b8

            def emit_w8_dma(unit):
                w8ts = []
                for oc in range(N_OC):
                    w8t = w8pool.tile([128, J, NB, 512], F8, tag="w8")
                    for j in range(J):
                        it = unit[j]
                        nc.sync.dma_start(
                            out=w8t[:, j, :, :],
                            in_=w8[it * 128:(it + 1) * 128,
                                   oc * WOC:(oc + 1) * WOC])
                    w8ts.append(w8t)
                return w8ts

            def emit_silu_mms(psums, sp, sbtts, start):
                # fp8 DoubleRow: the unit's two in-tiles form the k-pair
                for oc in range(N_OC):
                    for m in range(M_TILES):
                        nc.tensor.matmul(
                            psums[oc][m],
                            lhsT=sp[:, 0:2, m * 128:m * 128 + 128],
                            rhs=sbtts[oc],
                            start=start, stop=False, perf_mode=DR)

            def emit_chunk_mms(psums, b8, w8ts, oc, m, c0, c1, last_chunk,
                              last_unit):
                ms = slice(m * 128, m * 128 + 128)
                for j in range(J):
                    for gp in range(c0 // 2, c1 // 2):
                        nc.tensor.matmul(
                            psums[oc][m],
                            lhsT=b8[:, 2 * gp:2 * gp + 2, j, ms],
                            rhs=w8ts[oc][:, j, 2 * gp:2 * gp + 2, :],
                            start=False, stop=False, perf_mode=DR)
                if last_chunk:
                    # the two g=10 channels (j=0,1) as one k-pair
                    nc.tensor.matmul(
                        psums[oc][m], lhsT=b8[:, 10, 0:2, ms],
                        rhs=w8ts[oc][:, 0:2, 10, :],
                        start=False, stop=last_unit, perf_mode=DR)

            def emit_drain(psums, half, m):
                # ACT drains oc0, DVE drains oc1, into one full-row tile
                # so the y DMA moves contiguous 4KB rows
                t0 = half * HALF
                r0 = t0 + m * 128
                ot = opool.tile([128, 1024], F32, tag="out")
                nc.scalar.activation(ot[:, 0:512], psums[0][m], AF.Copy,
                                     bias=0.0, scale=1.0 / WSCALE)
                nc.vector.tensor_scalar(ot[:, 512:1024], psums[1][m],
                                        1.0 / WSCALE, 0.0, A.mult, A.add)
                for p in range(4):
                    nc.sync.dma_start(
                        out=y[r0 + 32 * p:r0 + 32 * (p + 1), :],
                        in_=ot[32 * p:32 * (p + 1), :])

            def emit_matmuls(psums, sp, sbtts, b8, w8ts, half, first_unit,
                             last_unit):
                emit_silu_mms(psums, sp, sbtts,
                              start=(first_unit and half == 1))
                if not last_unit:
                    # chunk-major: PE starts as soon as a chunk converts
                    for ci, (c0, c1) in enumerate(CH_CHUNKS):
                        lc = ci == len(CH_CHUNKS) - 1
                        for oc in range(N_OC):
                            for m in range(M_TILES):
                                emit_chunk_mms(psums, b8, w8ts, oc, m,
                                               c0, c1, lc, False)
                else:
                    # m-major: each bank finishes early; drain it while
                    # the remaining token tiles are still accumulating
                    for m in range(M_TILES):
                        for ci, (c0, c1) in enumerate(CH_CHUNKS):
                            lc = ci == len(CH_CHUNKS) - 1
                            for oc in range(N_OC):
                                emit_chunk_mms(psums, b8, w8ts, oc, m,
                                               c0, c1, lc, True)
                        emit_drain(psums, half, m)

            pre = None  # elementwise results pre-emitted for next half
            for half in range(2):
                psums = [[ppool.tile([128, 512], F32, tag="ps",
                                     name=f"ps_{half}_{_oc}_{_m}")
                          for _m in range(M_TILES)] for _oc in range(N_OC)]
                if half == 0:
                    # p-state warmup: zero-contribution matmuls during the
                    # DMA lead-in; they carry the half-0 start flags
                    banks = [(oc, m) for oc in range(N_OC)
                             for m in range(M_TILES)]
                    for wi in range(N_WARM):
                        oc, m = banks[wi % 8]
                        nc.tensor.matmul(psums[oc][m], lhsT=zl, rhs=zr,
                                         start=(wi < 8), stop=False)
                for ui, unit in enumerate(UNITS):
                    if half == 0:
                        emit_unit_dmas(ui, unit)
                    if ui == 0 and pre is not None:
                        sp, sbtts, b8 = pre
                        pre = None
                    else:
                        sp, sbtts, b8 = emit_elementwise(half, ui)
                    w8ts = emit_w8_dma(unit)
                    if half == 0 and ui == len(UNITS) - 1:
                        # pre-emit next half's first-unit elementwise so
                        # its ACT/DVE work overlaps this half's tail
                        pre = emit_elementwise(1, 0)
                    emit_matmuls(psums, sp, sbtts, b8, w8ts, half,
                                 ui == 0, ui == len(UNITS) - 1)
    nc.compile()
    return nc


def _prepare_inputs(x, coeff, scale_base, scale_spline):
    x = np.asarray(x, dtype=np.float32)
    coeff = np.asarray(coeff, dtype=np.float32)
    scale_base = np.asarray(scale_base, dtype=np.float32)
    ss = float(np.asarray(scale_spline).reshape(-1)[0])
    # w8[i, oc*5632 + g*512 + o] = coeff[oc*512+o, i, g] * ss * WSCALE/12
    w8 = (coeff * (ss * WSCALE / BSCALE)).transpose(1, 2, 0)  # [i, g, o]
    w8 = w8.reshape(IN_DIM, NB, N_OC, 512).transpose(0, 2, 1, 3)
    w8 = np.ascontiguousarray(w8).reshape(IN_DIM, NB * OUT_DIM)
    w8 = w8.astype(ml_dtypes.float8_e4m3)
    sbt = np.ascontiguousarray(scale_base.T * WSCALE).astype(
        ml_dtypes.float8_e4m3)
    in_maps = []
    for c in range(N_CORES):
        xtc = np.ascontiguousarray(x[c * TPC:(c + 1) * TPC, :].T).astype(
            np.float16)
        in_maps.append({"xt": xtc, "w8": w8, "sbt": sbt})
    return in_maps


def _get_bass():
    global _CACHED
    if _CACHED is None:
        _CACHED = _build_bass()
    return _CACHED


def run(inputs, trace=False, **kw):
    nc = _get_bass()
    in_maps = _prepare_inputs(inputs["x"], inputs["coeff"],
                              inputs["scale_base"], inputs["scale_spline"])
    res = run_bass_kernel_spmd(nc, in_maps, list(range(N_CORES)),
                               trace=trace, **kw)
    y = np.concatenate([np.asarray(res.results[c]["y"])
                        for c in range(N_CORES)], axis=0)
    return np.ascontiguousarray(y.astype(np.float32)), res


def kernel(x, grid, coeff, scale_base, scale_spline):
    y, _ = run({"x": x, "grid": grid, "coeff": coeff,
                "scale_base": scale_base, "scale_spline": scale_spline})
    return y
